# revision 10
# baseline (speedup 1.0000x reference)
"""Trainium2 Bass kernel for nn_ExaoneAttention (dense transformer attention).

Full-input contract: kernel(**inputs) takes the unsharded inputs and returns
the full [B, S, D] output. Internally shards across 8 NeuronCores:
2-way data parallel over batch x 4-way tensor parallel over kv heads
(2 kv heads = 8 query heads per core). Each core computes a partial
output through its Wo row-slice; the host sums the 4 partials per batch.

v2 design (vs the staged baseline):
- all matmul operands in bf16 (same PE rate, half DMA/SBUF, FWL loads)
- no DRAM staging round-trips: q/k/v/ctx stay resident in SBUF
- fused QKV projection; V is produced directly in [seq, hd] layout via
  seq-stationary matmuls (no PE transposes)
- softmax denominator accumulated on the PE (ones-matmul into a [1,512]
  PSUM row per head); causal diagonal narrowing; triangle mask applied
  as a [128,128] multiply after exp
- reciprocal reshaped to [128,4] via tiny DMA scatter/gather so it runs
  across lanes (the [1,512] single-lane reciprocal stalled PE 3.9us/head)
- output projection interleaved per query tile, Wo prefetched during
  attention
"""

import contextlib
import ctypes
import os
import sys
import types

import ml_dtypes
import numpy as np

# ---------------------------------------------------------------------------
# Problem constants (hardcoded per contract)
# ---------------------------------------------------------------------------
B, S, D = 2, 2048, 4096
H, HKV, HD = 32, 8, 128
G = H // HKV
THETA = 10000.0

NCORES = 8
BAT_SHARDS = 2
KV_SHARDS = 4
KVH = HKV // KV_SHARDS  # kv heads per core = 2
QH = KVH * G  # q heads per core = 8
FQ = QH * HD  # 1024
FKV = KVH * HD  # 256
DMC = D // 128  # 32 model-dim chunks

QT = 512  # query tile
NQT = S // QT  # 4
SC = 128  # sequence chunk
NSC = S // SC  # 16
DT = 512  # output d tile
NDT = D // DT  # 8

_SCALE = float(HD) ** -0.5

BF16 = ml_dtypes.bfloat16


# ---------------------------------------------------------------------------
# Wait-count legalization: this walrus build rejects instructions carrying
# more than a small number of sync waits (fused fp32/fp32r matmul: >1;
# drain: >4). Hoist excess waits onto standalone NoOps on the same engine
# immediately before the offending instruction; AND-semantics are preserved
# by sequential same-engine execution.
# ---------------------------------------------------------------------------
def _legalize_waits(nc):
    import bass_rust
    import concourse.mybir as mybir

    counter = 0
    for f in nc.m.functions:
        for bb in f.blocks:
            il = bb.instructions
            i = 0
            while i < len(il):
                ins = il[i]
                si = ins.sync_info
                if si is None or len(si.on_wait) <= 1:
                    i += 1
                    continue
                waits = list(si.on_wait)
                pos = i
                for w in waits[1:]:
                    counter += 1
                    nop = mybir.InstNoOp(name=f"lgw-{counter}", ins=[], outs=[])
                    nop.engine = ins.engine
                    nop.sync_info = bass_rust.SyncInfo(on_wait=[w], on_update=[])
                    il.insert(pos, nop)
                    pos += 1
                    i += 1
                ins.sync_info = bass_rust.SyncInfo(
                    on_wait=waits[:1], on_update=list(si.on_update)
                )
                i += 1
    return counter


# ---------------------------------------------------------------------------
# Bass kernel builder (per-core program; same program on all 8 cores)
# ---------------------------------------------------------------------------
def _build_nc():
    import concourse.bass as bass
    import concourse.mybir as mybir
    from concourse.tile import TileContext

    f32 = mybir.dt.float32
    f32r = mybir.dt.float32r
    bf16 = mybir.dt.bfloat16
    AF = mybir.ActivationFunctionType

    nc = bass.Bass()

    hiT = nc.declare_dram_parameter("hiT", [D, S], bf16, isOutput=False)
    wq = nc.declare_dram_parameter("wq", [D, FQ], bf16, isOutput=False)
    wk = nc.declare_dram_parameter("wk", [D, FKV], bf16, isOutput=False)
    wv = nc.declare_dram_parameter("wv", [D, FKV], bf16, isOutput=False)
    wo = nc.declare_dram_parameter("wo", [FQ, D], bf16, isOutput=False)
    ccT = nc.declare_dram_parameter("ccT", [HD, S], f32, isOutput=False)
    ssT = nc.declare_dram_parameter("ssT", [HD, S], f32, isOutput=False)
    tri = nc.declare_dram_parameter("tri", [SC, SC], bf16, isOutput=False)
    out = nc.declare_dram_parameter("out", [S, D], f32, isOutput=True)

    pred_d = nc.dram_tensor("pred_d", [2, QT], mybir.dt.float32)
    recip_d = nc.dram_tensor("recip_d", [2, QT], mybir.dt.float32r)

    hiT_r = hiT[:, :].rearrange("(c p) s -> p c s", p=128)
    wq_r = wq[:, :].rearrange("(c p) f -> p c f", p=128)
    wk_r = wk[:, :].rearrange("(c p) f -> p c f", p=128)
    wv_r = wv[:, :].rearrange("(c p) f -> p c f", p=128)
    wo_r = wo[:, :].rearrange("(h p) d -> p h d", p=128)

    with TileContext(nc) as tc, contextlib.ExitStack() as top:
        singles = top.enter_context(tc.tile_pool(name="singles", bufs=1))
        rope_pool = top.enter_context(tc.tile_pool(name="ropetmp", bufs=3))

        cc_sb = singles.tile([HD, S], f32)
        nc.sync.dma_start(out=cc_sb, in_=ccT[:, :])
        ss_sb = singles.tile([HD, S], f32)
        nc.sync.dma_start(out=ss_sb, in_=ssT[:, :])
        tri_sb = singles.tile([SC, SC], bf16)
        nc.sync.dma_start(out=tri_sb, in_=tri[:, :])
        ones_tmp = singles.tile([128, 128], f32)
        nc.vector.memset(ones_tmp, 1.0)
        ones_p = singles.tile([128, 1], bf16)
        nc.vector.tensor_copy(ones_p, ones_tmp[:, :1])
        ones_f = singles.tile([1, 128], f32r)
        nc.vector.tensor_copy(ones_f, ones_tmp[:1, :])

        # persistent activations (hd on partitions for q/k; seq for v)
        q_sb = singles.tile([128, QH, S], bf16)
        kt_sb = singles.tile([128, KVH, S], bf16)
        v_sb = singles.tile([128, KVH, NSC, HD], bf16)

        def rope_to(out_full, h_idx, ssl, psum):
            """roped = rope(psum); write bf16 into out_full[:, h_idx, ssl]."""
            cc = cc_sb[:, ssl]
            ss = ss_sb[:, ssl]
            t1 = rope_pool.tile([128, QT], f32, name="rope_t1")
            nc.vector.tensor_mul(t1, psum, cc)
            t2 = rope_pool.tile([128, QT], f32, name="rope_t2")
            nc.vector.tensor_mul(t2[:64], psum[64:], ss[:64])
            nc.vector.tensor_mul(t2[64:], psum[:64], ss[64:])
            nc.vector.tensor_sub(out_full[:64, h_idx, ssl], t1[:64], t2[:64])
            nc.vector.tensor_add(out_full[64:, h_idx, ssl], t1[64:], t2[64:])

        # ---------------- Phases A+B share the Wq prefetch pool -------------
        ab = top.enter_context(contextlib.ExitStack())
        wq_pool = ab.enter_context(tc.tile_pool(name="wqp", bufs=1))

        # ---------------- Phase A: K/V projection (hi chunk-streamed) -------
        with contextlib.ExitStack() as ph:
            wkv_pool = ph.enter_context(tc.tile_pool(name="wkv", bufs=1))
            hic_pool = ph.enter_context(tc.tile_pool(name="hic", bufs=6))
            ps_k = ph.enter_context(tc.tile_pool(name="ps_k", bufs=4, space="PSUM"))
            ps_v = ph.enter_context(tc.tile_pool(name="ps_v", bufs=4, space="PSUM"))

            wk_sb = wkv_pool.tile([128, DMC, FKV], bf16)
            nc.sync.dma_start(out=wk_sb, in_=wk_r)
            wv_sb = wkv_pool.tile([128, DMC, FKV], bf16)
            nc.sync.dma_start(out=wv_sb, in_=wv_r)
            wq_sb = None

            for st in range(NQT):
                ssl = slice(st * QT, (st + 1) * QT)
                pks = [ps_k.tile([128, QT], f32, name="pk") for _ in range(KVH)]
                pvs = [
                    ps_v.tile([128, KVH, HD], f32, name="pv")
                    for _ in range(QT // 128)
                ]
                for c in range(DMC):
                    hi_c = hic_pool.tile([128, QT], bf16, name="hi_c")
                    nc.sync.dma_start(out=hi_c, in_=hiT_r[:, c, ssl])
                    for kv in range(KVH):
                        nc.tensor.matmul(
                            pks[kv],
                            wk_sb[:, c, kv * 128 : (kv + 1) * 128],
                            hi_c,
                            start=(c == 0),
                            stop=(c == DMC - 1),
                        )
                    for j in range(QT // 128):
                        nc.tensor.matmul(
                            pvs[j],
                            hi_c[:, j * 128 : (j + 1) * 128],
                            wv_sb[:, c, :],
                            start=(c == 0),
                            stop=(c == DMC - 1),
                        )
                if st == 0:
                    # prefetch Wq now: behind phase A's first chunks in the
                    # DMA queue, done well before phase B needs it
                    wq_sb = wq_pool.tile([128, DMC, FQ], bf16)
                    nc.sync.dma_start(out=wq_sb, in_=wq_r)
                for kv in range(KVH):
                    rope_to(kt_sb, kv, ssl, pks[kv])
                for j in range(QT // 128):
                    nc.scalar.copy(v_sb[:, :, st * (QT // 128) + j, :], pvs[j])

        # ---------------- Phase B: Q projection (half-head passes) ----------
        with contextlib.ExitStack() as ph:
            hs_pool = ph.enter_context(tc.tile_pool(name="his", bufs=3))
            ps_q = ph.enter_context(tc.tile_pool(name="ps_q", bufs=8, space="PSUM"))
            HC = DMC // 2

            for st in range(NQT):
                ssl = slice(st * QT, (st + 1) * QT)
                slabs = []
                for hh in range(2):
                    slab = hs_pool.tile([128, HC, QT], bf16, name="hi_s")
                    nc.sync.dma_start(
                        out=slab, in_=hiT_r[:, hh * HC : (hh + 1) * HC, ssl]
                    )
                    slabs.append(slab)
                for hg in range(2):
                    banks = [
                        ps_q.tile([128, QT], f32, name="pq") for _ in range(QH // 2)
                    ]
                    for c in range(DMC):
                        for b, h in enumerate(range(hg * 4, hg * 4 + 4)):
                            nc.tensor.matmul(
                                banks[b],
                                wq_sb[:, c, h * 128 : (h + 1) * 128],
                                slabs[c // HC][:, c % HC, :],
                                start=(c == 0),
                                stop=(c == DMC - 1),
                            )
                    for b, h in enumerate(range(hg * 4, hg * 4 + 4)):
                        rope_to(q_sb, h, ssl, banks[b])

        ab.close()  # free Wq before the attention phase allocates

        # ---------------- Phase C+D: attention + output projection ----------
        with contextlib.ExitStack() as ph:
            wo_pool = ph.enter_context(tc.tile_pool(name="wop", bufs=1))
            ctx_pool = ph.enter_context(tc.tile_pool(name="ctxp", bufs=1))
            pt_pool = ph.enter_context(tc.tile_pool(name="ptp", bufs=4))
            ep_pool = ph.enter_context(tc.tile_pool(name="epp", bufs=2))
            bc_pool = ph.enter_context(tc.tile_pool(name="bcp", bufs=2))
            o_pool = ph.enter_context(tc.tile_pool(name="op", bufs=3))
            ps_a = ph.enter_context(tc.tile_pool(name="ps_a", bufs=4, space="PSUM"))
            ps_ctx = ph.enter_context(tc.tile_pool(name="ps_ctx", bufs=2, space="PSUM"))
            ps_pred = ph.enter_context(tc.tile_pool(name="ps_pred", bufs=2, space="PSUM"))

            wo_sb = wo_pool.tile([128, QH, D], bf16)
            nc.sync.dma_start(out=wo_sb, in_=wo_r)
            ctx_sb = ctx_pool.tile([128, QH, S], bf16)

            for qt in range(NQT):
                nk = G * (qt + 1)
                for h in range(QH):
                    kv = h // G
                    pctx = ps_ctx.tile([128, QT], f32, name="pctx")
                    ppred = ps_pred.tile([1, QT], f32, name="ppred")
                    for i in range(nk):
                        t = i - G * qt
                        qoff = t * 128 if t >= 0 else 0
                        pss = ps_a.tile([128, QT], f32, name="pss")
                        nc.tensor.matmul(
                            pss[:, qoff:],
                            kt_sb[:, kv, i * 128 : (i + 1) * 128],
                            q_sb[:, h, qt * QT + qoff : (qt + 1) * QT],
                            start=True,
                            stop=True,
                        )
                        pt = pt_pool.tile([128, QT], bf16, name="pt")
                        nc.scalar.activation(
                            pt[:, qoff:], pss[:, qoff:], AF.Exp, scale=_SCALE
                        )
                        if t >= 0:
                            nc.vector.tensor_mul(
                                pt[:, qoff : qoff + 128],
                                pt[:, qoff : qoff + 128],
                                tri_sb,
                            )
                        nc.tensor.matmul(
                            ppred[:, qoff:],
                            ones_p,
                            pt[:, qoff:],
                            start=(i == 0),
                            stop=(i == nk - 1),
                        )
                        nc.tensor.matmul(
                            pctx[:, qoff:],
                            v_sb[:, kv, i, :],
                            pt[:, qoff:],
                            start=(i == 0),
                            stop=(i == nk - 1),
                        )
                    # epilogue: denom -> [128,4] recip -> broadcast -> scale ctx
                    slot = h % 2
                    pred_sb = ep_pool.tile([1, QT], f32, name="pred_sb")
                    nc.scalar.copy(pred_sb, ppred)
                    nc.sync.dma_start(out=pred_d[slot, :], in_=pred_sb)
                    predT = ep_pool.tile([128, NQT], f32, name="predT")
                    nc.sync.dma_start(
                        out=predT,
                        in_=pred_d[slot, :].rearrange("(j p) -> p j", p=128),
                    )
                    recipT = ep_pool.tile([128, NQT], f32r, name="recipT")
                    with nc.allow_low_precision(reason="f32r recip: 1e-4 ok here"):
                        nc.vector.reciprocal(recipT, predT)
                    nc.sync.dma_start(
                        out=recip_d[slot, :].rearrange("(j p) -> p j", p=128),
                        in_=recipT,
                    )
                    rrow = ep_pool.tile([1, QT], f32r, name="rrow")
                    nc.sync.dma_start(out=rrow, in_=recip_d[slot, :])
                    pbc = ps_a.tile([128, QT], f32, name="pss")
                    nc.tensor.matmul(pbc, ones_f, rrow, start=True, stop=True)
                    bc_sb = bc_pool.tile([128, QT], f32, name="bc_sb")
                    nc.scalar.copy(bc_sb, pbc)
                    nc.vector.tensor_mul(
                        ctx_sb[:, h, qt * QT : (qt + 1) * QT], pctx, bc_sb
                    )

                # output projection for this query tile
                for j in range(QT // 128):
                    sc_idx = qt * (QT // 128) + j
                    for dt in range(NDT):
                        po = ps_a.tile([128, DT], f32, name="pss")
                        for h in range(QH):
                            nc.tensor.matmul(
                                po,
                                ctx_sb[:, h, sc_idx * 128 : (sc_idx + 1) * 128],
                                wo_sb[:, h, dt * DT : (dt + 1) * DT],
                                start=(h == 0),
                                stop=(h == QH - 1),
                            )
                        o_sb = o_pool.tile([128, DT], f32, name="o_sb")
                        nc.scalar.copy(o_sb, po)
                        nc.sync.dma_start(
                            out=out[
                                sc_idx * 128 : (sc_idx + 1) * 128,
                                dt * DT : (dt + 1) * DT,
                            ],
                            in_=o_sb,
                        )

    _legalize_waits(nc)
    return nc


_NC_CACHE = {}
_last_exec_ns = None


def _get_nc():
    if "nc" not in _NC_CACHE:
        _NC_CACHE["nc"] = _build_nc()
    return _NC_CACHE["nc"]


# ---------------------------------------------------------------------------
# Optional NTFF profiling hook (used by the local test harness via
# KERNEL_TRACE=1; grading path leaves it off)
# ---------------------------------------------------------------------------
def _install_ntff_hook(so_path="/opt/axon/libaxon_pjrt.so"):
    if "antenv.axon_hooks" in sys.modules:
        return
    try:
        lib = ctypes.CDLL(so_path)
    except OSError:
        lib = None
    if lib is None or not hasattr(lib, "axon_start_nrt_profile"):
        hook = None
    else:
        lib.axon_start_nrt_profile.argtypes = [
            ctypes.POINTER(ctypes.c_int64),
            ctypes.c_size_t,
        ]
        lib.axon_start_nrt_profile.restype = ctypes.c_int64
        lib.axon_stop_nrt_profile.argtypes = [ctypes.c_char_p]
        lib.axon_stop_nrt_profile.restype = ctypes.c_int64

        @contextlib.contextmanager
        def hook(output_dir, device_ids):
            import jax

            jax.devices()
            if device_ids:
                ids = (ctypes.c_int64 * len(device_ids))(*device_ids)
                rc = lib.axon_start_nrt_profile(ids, len(device_ids))
            else:
                rc = lib.axon_start_nrt_profile(None, 0)
            if rc != 0:
                raise RuntimeError(f"axon_start_nrt_profile rc={rc}")
            try:
                yield
            finally:
                n = lib.axon_stop_nrt_profile(str(output_dir).encode())
                print(f"ntff profile: {n} file(s) -> {output_dir}", file=sys.stderr)

    mod = types.ModuleType("antenv.axon_hooks")
    mod.get_axon_ntff_profile_hook = lambda: hook
    sys.modules["antenv.axon_hooks"] = mod


# ---------------------------------------------------------------------------
# Host entry point
# ---------------------------------------------------------------------------
def kernel(hidden_states, position_ids, attention_mask, Wq, Wk, Wv, Wo):
    global _last_exec_ns
    from concourse import bass_utils

    hidden_states = np.asarray(hidden_states, dtype=np.float32)
    position_ids = np.asarray(position_ids)
    attention_mask = np.asarray(attention_mask)
    Wq = np.asarray(Wq, dtype=np.float32)
    Wk = np.asarray(Wk, dtype=np.float32)
    Wv = np.asarray(Wv, dtype=np.float32)
    Wo = np.asarray(Wo, dtype=np.float32)

    if not np.all(np.asarray(attention_mask) > 0):
        # Spec guarantees an all-ones mask; fall back to a host reference
        # implementation for the general case rather than mis-computing.
        return _host_reference(
            hidden_states, position_ids, attention_mask, Wq, Wk, Wv, Wo
        )

    # rope tables per batch: cc/ss [HD, S] with halves stacked
    half = HD // 2
    inv_freq = 1.0 / (THETA ** (np.arange(0, half, dtype=np.float32) / half))
    ccs, sss = [], []
    for b in range(B):
        freqs = position_ids[b].astype(np.float32)[:, None] * inv_freq[None, :]
        cosT = np.cos(freqs).T.astype(np.float32)  # [64, S]
        sinT = np.sin(freqs).T.astype(np.float32)
        ccs.append(np.ascontiguousarray(np.concatenate([cosT, cosT], axis=0)))
        sss.append(np.ascontiguousarray(np.concatenate([sinT, sinT], axis=0)))

    # causal triangle for aligned diagonal 128-blocks: allowed iff q >= k
    kk = np.arange(SC)[:, None]
    qq = np.arange(SC)[None, :]
    tri = np.ascontiguousarray((qq >= kk).astype(BF16))

    hiTs = [np.ascontiguousarray(hidden_states[b].T.astype(BF16)) for b in range(B)]

    in_maps = []
    for c in range(NCORES):
        b = c // KV_SHARDS
        m = c % KV_SHARDS
        qcols = slice(m * FQ, (m + 1) * FQ)
        kvcols = slice(m * FKV, (m + 1) * FKV)
        in_maps.append(
            {
                "hiT": hiTs[b],
                "wq": np.ascontiguousarray(Wq[:, qcols].astype(BF16)),
                "wk": np.ascontiguousarray(Wk[:, kvcols].astype(BF16)),
                "wv": np.ascontiguousarray(Wv[:, kvcols].astype(BF16)),
                "wo": np.ascontiguousarray(Wo[qcols, :].astype(BF16)),
                "ccT": ccs[b],
                "ssT": sss[b],
                "tri": tri,
            }
        )

    nc = _get_nc()
    trace = os.environ.get("KERNEL_TRACE", "") == "1"
    if trace:
        _install_ntff_hook()
        bass_utils.upload_artifacts = lambda tmpdir: f"local:{tmpdir}"
    res = bass_utils.run_bass_kernel_spmd(
        nc, in_maps, list(range(NCORES)), trace=trace
    )
    _last_exec_ns = res.exec_time_ns

    out = np.zeros((B, S, D), dtype=np.float32)
    for c in range(NCORES):
        out[c // KV_SHARDS] += res.results[c]["out"]
    return out


def _host_reference(hidden_states, position_ids, attention_mask, Wq, Wk, Wv, Wo):
    """Numpy fallback for inputs outside the spec's guarantees."""
    q = (hidden_states @ Wq).reshape(B, S, H, HD)
    k = (hidden_states @ Wk).reshape(B, S, HKV, HD)
    v = (hidden_states @ Wv).reshape(B, S, HKV, HD)

    half = HD // 2
    inv_freq = 1.0 / (THETA ** (np.arange(0, half, dtype=np.float32) / half))
    freqs = position_ids.astype(np.float32)[..., None] * inv_freq
    cos = np.cos(freqs)[:, :, None, :]
    sin = np.sin(freqs)[:, :, None, :]

    def rope(x):
        x1, x2 = x[..., :half], x[..., half:]
        return np.concatenate([x1 * cos - x2 * sin, x2 * cos + x1 * sin], axis=-1)

    q, k = rope(q), rope(k)
    qg = q.reshape(B, S, HKV, G, HD)
    scores = np.einsum("bqhgd,bkhd->bhgqk", qg, k) * (HD**-0.5)
    causal = np.tril(np.ones((S, S), bool))
    mask = causal[None, None, None] & (attention_mask[:, None, None, None, :] > 0)
    scores = np.where(mask, scores, np.finfo(np.float32).min)
    scores = scores - scores.max(axis=-1, keepdims=True)
    probs = np.exp(scores)
    probs = probs / probs.sum(axis=-1, keepdims=True)
    ctx = np.einsum("bhgqk,bkhd->bqhgd", probs, v).reshape(B, S, H * HD)
    return (ctx @ Wo).astype(np.float32)


# revision 13
# speedup vs baseline: 1.4159x; 1.4159x over previous
"""Trainium2 Bass kernel for nn_ExaoneAttention (dense transformer attention).

Full-input contract: kernel(**inputs) takes the unsharded inputs and returns
the full [B, S, D] output. Internally shards across 8 NeuronCores:
2-way data parallel over batch x 4-way tensor parallel over kv heads
(2 kv heads = 8 query heads per core). Each core computes a partial
output through its Wo row-slice; the host sums the 4 partials per batch.

v2 design (vs the staged baseline):
- all matmul operands in bf16 (same PE rate, half DMA/SBUF, FWL loads)
- no DRAM staging round-trips: q/k/v/ctx stay resident in SBUF
- fused QKV projection; V is produced directly in [seq, hd] layout via
  seq-stationary matmuls (no PE transposes)
- softmax denominator accumulated on the PE (ones-matmul into a [1,512]
  PSUM row per head); causal diagonal narrowing; triangle mask applied
  as a [128,128] multiply after exp
- reciprocal reshaped to [128,4] via tiny DMA scatter/gather so it runs
  across lanes (the [1,512] single-lane reciprocal stalled PE 3.9us/head)
- output projection interleaved per query tile, Wo prefetched during
  attention
"""

import contextlib
import ctypes
import os
import sys
import types

import ml_dtypes
import numpy as np

# ---------------------------------------------------------------------------
# Problem constants (hardcoded per contract)
# ---------------------------------------------------------------------------
B, S, D = 2, 2048, 4096
H, HKV, HD = 32, 8, 128
G = H // HKV
THETA = 10000.0

NCORES = 8
BAT_SHARDS = 2
KV_SHARDS = 4
KVH = HKV // KV_SHARDS  # kv heads per core = 2
QH = KVH * G  # q heads per core = 8
FQ = QH * HD  # 1024
FKV = KVH * HD  # 256
DMC = D // 128  # 32 model-dim chunks

QT = 512  # query tile
NQT = S // QT  # 4
SC = 128  # sequence chunk
NSC = S // SC  # 16
DT = 512  # output d tile
NDT = D // DT  # 8

_SCALE = float(HD) ** -0.5

BF16 = ml_dtypes.bfloat16


# ---------------------------------------------------------------------------
# Wait-count legalization: this walrus build rejects instructions carrying
# more than a small number of sync waits (fused fp32/fp32r matmul: >1;
# drain: >4). Hoist excess waits onto standalone NoOps on the same engine
# immediately before the offending instruction; AND-semantics are preserved
# by sequential same-engine execution.
# ---------------------------------------------------------------------------
def _legalize_waits(nc):
    import bass_rust
    import concourse.mybir as mybir

    counter = 0
    for f in nc.m.functions:
        for bb in f.blocks:
            il = bb.instructions
            i = 0
            while i < len(il):
                ins = il[i]
                si = ins.sync_info
                if si is None or len(si.on_wait) <= 1:
                    i += 1
                    continue
                waits = list(si.on_wait)
                pos = i
                for w in waits[1:]:
                    counter += 1
                    nop = mybir.InstNoOp(name=f"lgw-{counter}", ins=[], outs=[])
                    nop.engine = ins.engine
                    nop.sync_info = bass_rust.SyncInfo(on_wait=[w], on_update=[])
                    il.insert(pos, nop)
                    pos += 1
                    i += 1
                ins.sync_info = bass_rust.SyncInfo(
                    on_wait=waits[:1], on_update=list(si.on_update)
                )
                i += 1
    return counter


# ---------------------------------------------------------------------------
# Bass kernel builder (per-core program; same program on all 8 cores)
# ---------------------------------------------------------------------------
def _build_nc():
    import concourse.bass as bass
    import concourse.mybir as mybir
    from concourse.tile import TileContext

    f32 = mybir.dt.float32
    f32r = mybir.dt.float32r
    bf16 = mybir.dt.bfloat16
    AF = mybir.ActivationFunctionType

    nc = bass.Bass()

    hiT = nc.declare_dram_parameter("hiT", [D, S], bf16, isOutput=False)
    wq = nc.declare_dram_parameter("wq", [D, FQ], bf16, isOutput=False)
    wk = nc.declare_dram_parameter("wk", [D, FKV], bf16, isOutput=False)
    wv = nc.declare_dram_parameter("wv", [D, FKV], bf16, isOutput=False)
    wo = nc.declare_dram_parameter("wo", [FQ, D], bf16, isOutput=False)
    ccT = nc.declare_dram_parameter("ccT", [HD, S], f32, isOutput=False)
    ssT = nc.declare_dram_parameter("ssT", [HD, S], f32, isOutput=False)
    tri = nc.declare_dram_parameter("tri", [SC, SC], bf16, isOutput=False)
    out = nc.declare_dram_parameter("out", [S, D], f32, isOutput=True)

    pred_d = nc.dram_tensor("pred_d", [4, QT], mybir.dt.float32)
    recip_d = nc.dram_tensor("recip_d", [4, QT], mybir.dt.float32r)

    hiT_r = hiT[:, :].rearrange("(c p) s -> p c s", p=128)
    wq_r = wq[:, :].rearrange("(c p) f -> p c f", p=128)
    wk_r = wk[:, :].rearrange("(c p) f -> p c f", p=128)
    wv_r = wv[:, :].rearrange("(c p) f -> p c f", p=128)
    wo_r = wo[:, :].rearrange("(h p) d -> p h d", p=128)

    with TileContext(nc) as tc, contextlib.ExitStack() as top:
        singles = top.enter_context(tc.tile_pool(name="singles", bufs=1))
        rope_pool = top.enter_context(tc.tile_pool(name="ropetmp", bufs=3))

        cc_sb = singles.tile([HD, S], f32)
        ss_sb = singles.tile([HD, S], f32)
        tri_sb = singles.tile([SC, SC], bf16)
        ones_tmp = singles.tile([128, 128], f32)
        nc.vector.memset(ones_tmp, 1.0)
        ones_p = singles.tile([128, 1], bf16)
        nc.vector.tensor_copy(ones_p, ones_tmp[:, :1])
        ones_f = singles.tile([1, 128], f32r)
        nc.vector.tensor_copy(ones_f, ones_tmp[:1, :])

        # persistent activations (hd on partitions for q/k; seq for v)
        q_sb = singles.tile([128, QH, S], bf16)
        kt_sb = singles.tile([128, KVH, S], bf16)
        v_sb = singles.tile([128, KVH, NSC, HD], bf16)

        def rope_to(out_full, h_idx, ssl, psum):
            """roped = rope(psum); write bf16 into out_full[:, h_idx, ssl]."""
            cc = cc_sb[:, ssl]
            ss = ss_sb[:, ssl]
            t1 = rope_pool.tile([128, QT], f32, name="rope_t1")
            nc.vector.tensor_mul(t1, psum, cc)
            t2 = rope_pool.tile([128, QT], f32, name="rope_t2")
            nc.vector.tensor_mul(t2[:64], psum[64:], ss[:64])
            nc.vector.tensor_mul(t2[64:], psum[:64], ss[64:])
            nc.vector.tensor_sub(out_full[:64, h_idx, ssl], t1[:64], t2[:64])
            nc.vector.tensor_add(out_full[64:, h_idx, ssl], t1[64:], t2[64:])

        # ---------------- Phases A+B share the Wq prefetch pool -------------
        ab = top.enter_context(contextlib.ExitStack())
        wq_pool = ab.enter_context(tc.tile_pool(name="wqp", bufs=1))

        # ---------------- Phase A: K/V projection (hi chunk-streamed) -------
        with contextlib.ExitStack() as ph:
            wkv_pool = ph.enter_context(tc.tile_pool(name="wkv", bufs=1))
            hic_pool = ph.enter_context(tc.tile_pool(name="hic", bufs=6))
            ps_k = ph.enter_context(tc.tile_pool(name="ps_k", bufs=4, space="PSUM"))
            ps_v = ph.enter_context(tc.tile_pool(name="ps_v", bufs=4, space="PSUM"))

            wk_sb = wkv_pool.tile([128, DMC, FKV], bf16)
            wv_sb = wkv_pool.tile([128, DMC, FKV], bf16)
            wq_sb = None

            for st in range(NQT):
                ssl = slice(st * QT, (st + 1) * QT)
                pks = [ps_k.tile([128, QT], f32, name="pk") for _ in range(KVH)]
                pvs = [
                    ps_v.tile([128, KVH, HD], f32, name="pv")
                    for _ in range(QT // 128)
                ]
                for c in range(DMC):
                    if st == 0:
                        nc.scalar.dma_start(out=wk_sb[:, c, :], in_=wk_r[:, c, :])
                        nc.scalar.dma_start(out=wv_sb[:, c, :], in_=wv_r[:, c, :])
                    hi_c = hic_pool.tile([128, QT], bf16, name="hi_c")
                    nc.sync.dma_start(out=hi_c, in_=hiT_r[:, c, ssl])
                    for kv in range(KVH):
                        nc.tensor.matmul(
                            pks[kv],
                            wk_sb[:, c, kv * 128 : (kv + 1) * 128],
                            hi_c,
                            start=(c == 0),
                            stop=(c == DMC - 1),
                        )
                    for j in range(QT // 128):
                        nc.tensor.matmul(
                            pvs[j],
                            hi_c[:, j * 128 : (j + 1) * 128],
                            wv_sb[:, c, :],
                            start=(c == 0),
                            stop=(c == DMC - 1),
                        )
                if st == 0:
                    # rope tables + Wq prefetch ride the scalar-engine DMA
                    # queue so they never block the hi-chunk stream
                    nc.scalar.dma_start(out=cc_sb, in_=ccT[:, :])
                    nc.scalar.dma_start(out=ss_sb, in_=ssT[:, :])
                    nc.scalar.dma_start(out=tri_sb, in_=tri[:, :])
                    wq_sb = wq_pool.tile([128, DMC, FQ], bf16)
                    nc.scalar.dma_start(out=wq_sb, in_=wq_r)
                for kv in range(KVH):
                    rope_to(kt_sb, kv, ssl, pks[kv])
                for j in range(QT // 128):
                    nc.scalar.copy(v_sb[:, :, st * (QT // 128) + j, :], pvs[j])

        # ---------------- Phase B: Q projection (half-head passes) ----------
        with contextlib.ExitStack() as ph:
            hs_pool = ph.enter_context(tc.tile_pool(name="his", bufs=3))
            ps_q = ph.enter_context(tc.tile_pool(name="ps_q", bufs=8, space="PSUM"))
            HC = DMC // 2

            for st in range(NQT):
                ssl = slice(st * QT, (st + 1) * QT)
                slabs = []
                for hh in range(2):
                    slab = hs_pool.tile([128, HC, QT], bf16, name="hi_s")
                    nc.sync.dma_start(
                        out=slab, in_=hiT_r[:, hh * HC : (hh + 1) * HC, ssl]
                    )
                    slabs.append(slab)
                for hg in range(2):
                    banks = [
                        ps_q.tile([128, QT], f32, name="pq") for _ in range(QH // 2)
                    ]
                    for c in range(DMC):
                        for b, h in enumerate(range(hg * 4, hg * 4 + 4)):
                            nc.tensor.matmul(
                                banks[b],
                                wq_sb[:, c, h * 128 : (h + 1) * 128],
                                slabs[c // HC][:, c % HC, :],
                                start=(c == 0),
                                stop=(c == DMC - 1),
                            )
                    for b, h in enumerate(range(hg * 4, hg * 4 + 4)):
                        rope_to(q_sb, h, ssl, banks[b])

        ab.close()  # free Wq before the attention phase allocates

        # ---------------- Phase C+D: attention + output projection ----------
        # The per-head normalization chain (pred -> DRAM transpose -> recip
        # -> broadcast matmul -> scale) has ~15us of DMA latency. Emitting it
        # inline stalls the PE, so each head's tail (broadcast + normalize)
        # is deferred ~2 heads via a closure queue; ctx is evacuated
        # unnormalized to SBUF so its PSUM bank frees immediately.
        with contextlib.ExitStack() as ph:
            wo_pool = ph.enter_context(tc.tile_pool(name="wop", bufs=1))
            ctx_pool = ph.enter_context(tc.tile_pool(name="ctxp", bufs=1))
            pt_pool = ph.enter_context(tc.tile_pool(name="ptp", bufs=4))
            ep_pool = ph.enter_context(tc.tile_pool(name="epp", bufs=4))
            cu_pool = ph.enter_context(tc.tile_pool(name="cup", bufs=4))
            bc_pool = ph.enter_context(tc.tile_pool(name="bcp", bufs=2))
            o_pool = ph.enter_context(tc.tile_pool(name="op", bufs=3))
            ps_a = ph.enter_context(tc.tile_pool(name="ps_a", bufs=4, space="PSUM"))
            ps_ctx = ph.enter_context(tc.tile_pool(name="ps_ctx", bufs=2, space="PSUM"))
            ps_pred = ph.enter_context(tc.tile_pool(name="ps_pred", bufs=2, space="PSUM"))

            wo_sb = wo_pool.tile([128, QH, D], bf16)
            nc.scalar.dma_start(out=wo_sb, in_=wo_r)
            ctx_sb = ctx_pool.tile([128, QH, S], bf16)

            deferred = []

            def emit_head(qt, h):
                nk = G * (qt + 1)
                kv = h // G
                pctx = ps_ctx.tile([128, QT], f32, name="pctx")
                ppred = ps_pred.tile([1, QT], f32, name="ppred")
                for i in range(nk):
                    t = i - G * qt
                    qoff = t * 128 if t >= 0 else 0
                    pss = ps_a.tile([128, QT], f32, name="pss")
                    nc.tensor.matmul(
                        pss[:, qoff:],
                        kt_sb[:, kv, i * 128 : (i + 1) * 128],
                        q_sb[:, h, qt * QT + qoff : (qt + 1) * QT],
                        start=True,
                        stop=True,
                    )
                    pt = pt_pool.tile([128, QT], bf16, name="pt")
                    nc.scalar.activation(
                        pt[:, qoff:], pss[:, qoff:], AF.Exp, scale=_SCALE
                    )
                    if t >= 0:
                        nc.vector.tensor_mul(
                            pt[:, qoff : qoff + 128],
                            pt[:, qoff : qoff + 128],
                            tri_sb,
                        )
                    nc.tensor.matmul(
                        ppred[:, qoff:],
                        ones_p,
                        pt[:, qoff:],
                        start=(i == 0),
                        stop=(i == nk - 1),
                    )
                    nc.tensor.matmul(
                        pctx[:, qoff:],
                        v_sb[:, kv, i, :],
                        pt[:, qoff:],
                        start=(i == 0),
                        stop=(i == nk - 1),
                    )
                # evacuate unnormalized ctx; launch denom->recip chain
                ctxu = cu_pool.tile([128, QT], bf16, name="ctxu")
                nc.scalar.copy(ctxu, pctx)
                slot = (qt * QH + h) % 4
                pred_sb = ep_pool.tile([1, QT], f32, name="pred_sb")
                nc.scalar.copy(pred_sb, ppred)
                nc.sync.dma_start(out=pred_d[slot, :], in_=pred_sb)
                predT = ep_pool.tile([128, NQT], f32, name="predT")
                nc.sync.dma_start(
                    out=predT,
                    in_=pred_d[slot, :].rearrange("(j p) -> p j", p=128),
                )
                recipT = ep_pool.tile([128, NQT], f32r, name="recipT")
                with nc.allow_low_precision(reason="f32r recip: 1e-4 ok here"):
                    nc.vector.reciprocal(recipT, predT)
                nc.sync.dma_start(
                    out=recip_d[slot, :].rearrange("(j p) -> p j", p=128),
                    in_=recipT,
                )
                rrow = ep_pool.tile([1, QT], f32r, name="rrow")
                nc.sync.dma_start(out=rrow, in_=recip_d[slot, :])

                def tail():
                    pbc = ps_a.tile([128, QT], f32, name="pss")
                    nc.tensor.matmul(pbc, ones_f, rrow, start=True, stop=True)
                    bc_sb = bc_pool.tile([128, QT], bf16, name="bc_sb")
                    nc.scalar.copy(bc_sb, pbc)
                    nc.vector.tensor_mul(
                        ctx_sb[:, h, qt * QT : (qt + 1) * QT], ctxu, bc_sb
                    )

                return tail

            def emit_outproj(qt):
                def run():
                    for j in range(QT // 128):
                        sc_idx = qt * (QT // 128) + j
                        for dt in range(NDT):
                            po = ps_a.tile([128, DT], f32, name="pss")
                            for h in range(QH):
                                nc.tensor.matmul(
                                    po,
                                    ctx_sb[:, h, sc_idx * 128 : (sc_idx + 1) * 128],
                                    wo_sb[:, h, dt * DT : (dt + 1) * DT],
                                    start=(h == 0),
                                    stop=(h == QH - 1),
                                )
                            o_sb = o_pool.tile([128, DT], f32, name="o_sb")
                            nc.scalar.copy(o_sb, po)
                            nc.sync.dma_start(
                                out=out[
                                    sc_idx * 128 : (sc_idx + 1) * 128,
                                    dt * DT : (dt + 1) * DT,
                                ],
                                in_=o_sb,
                            )

                return run

            for qt in range(NQT):
                for h in range(QH):
                    deferred.append(emit_head(qt, h))
                    while len(deferred) > 2:
                        deferred.pop(0)()
                deferred.append(emit_outproj(qt))
            while deferred:
                deferred.pop(0)()

    _legalize_waits(nc)
    return nc


_NC_CACHE = {}
_last_exec_ns = None


def _get_nc():
    if "nc" not in _NC_CACHE:
        _NC_CACHE["nc"] = _build_nc()
    return _NC_CACHE["nc"]


# ---------------------------------------------------------------------------
# Optional NTFF profiling hook (used by the local test harness via
# KERNEL_TRACE=1; grading path leaves it off)
# ---------------------------------------------------------------------------
def _install_ntff_hook(so_path="/opt/axon/libaxon_pjrt.so"):
    if "antenv.axon_hooks" in sys.modules:
        return
    try:
        lib = ctypes.CDLL(so_path)
    except OSError:
        lib = None
    if lib is None or not hasattr(lib, "axon_start_nrt_profile"):
        hook = None
    else:
        lib.axon_start_nrt_profile.argtypes = [
            ctypes.POINTER(ctypes.c_int64),
            ctypes.c_size_t,
        ]
        lib.axon_start_nrt_profile.restype = ctypes.c_int64
        lib.axon_stop_nrt_profile.argtypes = [ctypes.c_char_p]
        lib.axon_stop_nrt_profile.restype = ctypes.c_int64

        @contextlib.contextmanager
        def hook(output_dir, device_ids):
            import jax

            jax.devices()
            if device_ids:
                ids = (ctypes.c_int64 * len(device_ids))(*device_ids)
                rc = lib.axon_start_nrt_profile(ids, len(device_ids))
            else:
                rc = lib.axon_start_nrt_profile(None, 0)
            if rc != 0:
                raise RuntimeError(f"axon_start_nrt_profile rc={rc}")
            try:
                yield
            finally:
                n = lib.axon_stop_nrt_profile(str(output_dir).encode())
                print(f"ntff profile: {n} file(s) -> {output_dir}", file=sys.stderr)

    mod = types.ModuleType("antenv.axon_hooks")
    mod.get_axon_ntff_profile_hook = lambda: hook
    sys.modules["antenv.axon_hooks"] = mod


# ---------------------------------------------------------------------------
# Host entry point
# ---------------------------------------------------------------------------
def kernel(hidden_states, position_ids, attention_mask, Wq, Wk, Wv, Wo):
    global _last_exec_ns
    from concourse import bass_utils

    hidden_states = np.asarray(hidden_states, dtype=np.float32)
    position_ids = np.asarray(position_ids)
    attention_mask = np.asarray(attention_mask)
    Wq = np.asarray(Wq, dtype=np.float32)
    Wk = np.asarray(Wk, dtype=np.float32)
    Wv = np.asarray(Wv, dtype=np.float32)
    Wo = np.asarray(Wo, dtype=np.float32)

    if not np.all(np.asarray(attention_mask) > 0):
        # Spec guarantees an all-ones mask; fall back to a host reference
        # implementation for the general case rather than mis-computing.
        return _host_reference(
            hidden_states, position_ids, attention_mask, Wq, Wk, Wv, Wo
        )

    # rope tables per batch: cc/ss [HD, S] with halves stacked
    half = HD // 2
    inv_freq = 1.0 / (THETA ** (np.arange(0, half, dtype=np.float32) / half))
    ccs, sss = [], []
    for b in range(B):
        freqs = position_ids[b].astype(np.float32)[:, None] * inv_freq[None, :]
        cosT = np.cos(freqs).T.astype(np.float32)  # [64, S]
        sinT = np.sin(freqs).T.astype(np.float32)
        ccs.append(np.ascontiguousarray(np.concatenate([cosT, cosT], axis=0)))
        sss.append(np.ascontiguousarray(np.concatenate([sinT, sinT], axis=0)))

    # causal triangle for aligned diagonal 128-blocks: allowed iff q >= k
    kk = np.arange(SC)[:, None]
    qq = np.arange(SC)[None, :]
    tri = np.ascontiguousarray((qq >= kk).astype(BF16))

    hiTs = [np.ascontiguousarray(hidden_states[b].T.astype(BF16)) for b in range(B)]

    in_maps = []
    for c in range(NCORES):
        b = c // KV_SHARDS
        m = c % KV_SHARDS
        qcols = slice(m * FQ, (m + 1) * FQ)
        kvcols = slice(m * FKV, (m + 1) * FKV)
        in_maps.append(
            {
                "hiT": hiTs[b],
                "wq": np.ascontiguousarray(Wq[:, qcols].astype(BF16)),
                "wk": np.ascontiguousarray(Wk[:, kvcols].astype(BF16)),
                "wv": np.ascontiguousarray(Wv[:, kvcols].astype(BF16)),
                "wo": np.ascontiguousarray(Wo[qcols, :].astype(BF16)),
                "ccT": ccs[b],
                "ssT": sss[b],
                "tri": tri,
            }
        )

    nc = _get_nc()
    trace = os.environ.get("KERNEL_TRACE", "") == "1"
    if trace:
        _install_ntff_hook()
        bass_utils.upload_artifacts = lambda tmpdir: f"local:{tmpdir}"
    res = bass_utils.run_bass_kernel_spmd(
        nc, in_maps, list(range(NCORES)), trace=trace
    )
    _last_exec_ns = res.exec_time_ns

    out = np.zeros((B, S, D), dtype=np.float32)
    for c in range(NCORES):
        out[c // KV_SHARDS] += res.results[c]["out"]
    return out


def _host_reference(hidden_states, position_ids, attention_mask, Wq, Wk, Wv, Wo):
    """Numpy fallback for inputs outside the spec's guarantees."""
    q = (hidden_states @ Wq).reshape(B, S, H, HD)
    k = (hidden_states @ Wk).reshape(B, S, HKV, HD)
    v = (hidden_states @ Wv).reshape(B, S, HKV, HD)

    half = HD // 2
    inv_freq = 1.0 / (THETA ** (np.arange(0, half, dtype=np.float32) / half))
    freqs = position_ids.astype(np.float32)[..., None] * inv_freq
    cos = np.cos(freqs)[:, :, None, :]
    sin = np.sin(freqs)[:, :, None, :]

    def rope(x):
        x1, x2 = x[..., :half], x[..., half:]
        return np.concatenate([x1 * cos - x2 * sin, x2 * cos + x1 * sin], axis=-1)

    q, k = rope(q), rope(k)
    qg = q.reshape(B, S, HKV, G, HD)
    scores = np.einsum("bqhgd,bkhd->bhgqk", qg, k) * (HD**-0.5)
    causal = np.tril(np.ones((S, S), bool))
    mask = causal[None, None, None] & (attention_mask[:, None, None, None, :] > 0)
    scores = np.where(mask, scores, np.finfo(np.float32).min)
    scores = scores - scores.max(axis=-1, keepdims=True)
    probs = np.exp(scores)
    probs = probs / probs.sum(axis=-1, keepdims=True)
    ctx = np.einsum("bhgqk,bkhd->bqhgd", probs, v).reshape(B, S, H * HD)
    return (ctx @ Wo).astype(np.float32)


# revision 15
# speedup vs baseline: 1.4592x; 1.0306x over previous
"""Trainium2 Bass kernel for nn_ExaoneAttention (dense transformer attention).

Full-input contract: kernel(**inputs) takes the unsharded inputs and returns
the full [B, S, D] output. Internally shards across 8 NeuronCores:
2-way data parallel over batch x 4-way tensor parallel over kv heads
(2 kv heads = 8 query heads per core). Each core computes a partial
output through its Wo row-slice; the host sums the 4 partials per batch.

v2 design (vs the staged baseline):
- all matmul operands in bf16 (same PE rate, half DMA/SBUF, FWL loads)
- no DRAM staging round-trips: q/k/v/ctx stay resident in SBUF
- fused QKV projection; V is produced directly in [seq, hd] layout via
  seq-stationary matmuls (no PE transposes)
- softmax denominator accumulated on the PE (ones-matmul into a [1,512]
  PSUM row per head); causal diagonal narrowing; triangle mask applied
  as a [128,128] multiply after exp
- reciprocal reshaped to [128,4] via tiny DMA scatter/gather so it runs
  across lanes (the [1,512] single-lane reciprocal stalled PE 3.9us/head)
- output projection interleaved per query tile, Wo prefetched during
  attention
"""

import contextlib
import ctypes
import os
import sys
import types

import ml_dtypes
import numpy as np

# ---------------------------------------------------------------------------
# Problem constants (hardcoded per contract)
# ---------------------------------------------------------------------------
B, S, D = 2, 2048, 4096
H, HKV, HD = 32, 8, 128
G = H // HKV
THETA = 10000.0

NCORES = 8
BAT_SHARDS = 2
KV_SHARDS = 4
KVH = HKV // KV_SHARDS  # kv heads per core = 2
QH = KVH * G  # q heads per core = 8
FQ = QH * HD  # 1024
FKV = KVH * HD  # 256
DMC = D // 128  # 32 model-dim chunks

QT = 512  # query tile
NQT = S // QT  # 4
SC = 128  # sequence chunk
NSC = S // SC  # 16
DT = 512  # output d tile
NDT = D // DT  # 8

_SCALE = float(HD) ** -0.5

BF16 = ml_dtypes.bfloat16


# ---------------------------------------------------------------------------
# Wait-count legalization: this walrus build rejects instructions carrying
# more than a small number of sync waits (fused fp32/fp32r matmul: >1;
# drain: >4). Hoist excess waits onto standalone NoOps on the same engine
# immediately before the offending instruction; AND-semantics are preserved
# by sequential same-engine execution.
# ---------------------------------------------------------------------------
def _legalize_waits(nc):
    import bass_rust
    import concourse.mybir as mybir

    counter = 0
    for f in nc.m.functions:
        for bb in f.blocks:
            il = bb.instructions
            i = 0
            while i < len(il):
                ins = il[i]
                si = ins.sync_info
                if si is None or len(si.on_wait) <= 1:
                    i += 1
                    continue
                waits = list(si.on_wait)
                pos = i
                for w in waits[1:]:
                    counter += 1
                    nop = mybir.InstNoOp(name=f"lgw-{counter}", ins=[], outs=[])
                    nop.engine = ins.engine
                    nop.sync_info = bass_rust.SyncInfo(on_wait=[w], on_update=[])
                    il.insert(pos, nop)
                    pos += 1
                    i += 1
                ins.sync_info = bass_rust.SyncInfo(
                    on_wait=waits[:1], on_update=list(si.on_update)
                )
                i += 1
    return counter


# ---------------------------------------------------------------------------
# Bass kernel builder (per-core program; same program on all 8 cores)
# ---------------------------------------------------------------------------
def _build_nc():
    import concourse.bass as bass
    import concourse.mybir as mybir
    from concourse.tile import TileContext

    f32 = mybir.dt.float32
    f32r = mybir.dt.float32r
    bf16 = mybir.dt.bfloat16
    AF = mybir.ActivationFunctionType

    nc = bass.Bass()

    hiT = nc.declare_dram_parameter("hiT", [D, S], bf16, isOutput=False)
    wq = nc.declare_dram_parameter("wq", [D, FQ], bf16, isOutput=False)
    wk = nc.declare_dram_parameter("wk", [D, FKV], bf16, isOutput=False)
    wv = nc.declare_dram_parameter("wv", [D, FKV], bf16, isOutput=False)
    wo = nc.declare_dram_parameter("wo", [FQ, D], bf16, isOutput=False)
    ccT = nc.declare_dram_parameter("ccT", [HD, S], f32, isOutput=False)
    ssT = nc.declare_dram_parameter("ssT", [HD, S], f32, isOutput=False)
    tri = nc.declare_dram_parameter("tri", [SC, SC], bf16, isOutput=False)
    out = nc.declare_dram_parameter("out", [S, D], f32, isOutput=True)

    pred_d = nc.dram_tensor("pred_d", [4, QT], mybir.dt.float32)
    recip_d = nc.dram_tensor("recip_d", [4, QT], mybir.dt.float32r)

    hiT_r = hiT[:, :].rearrange("(c p) s -> p c s", p=128)
    wq_r = wq[:, :].rearrange("(c p) f -> p c f", p=128)
    wk_r = wk[:, :].rearrange("(c p) f -> p c f", p=128)
    wv_r = wv[:, :].rearrange("(c p) f -> p c f", p=128)
    wo_r = wo[:, :].rearrange("(h p) d -> p h d", p=128)

    with TileContext(nc) as tc, contextlib.ExitStack() as top:
        singles = top.enter_context(tc.tile_pool(name="singles", bufs=1))

        cc_sb = singles.tile([HD, S], f32)
        ss_sb = singles.tile([HD, S], f32)
        tri_sb = singles.tile([SC, SC], bf16)
        ones_tmp = singles.tile([128, 128], f32)
        nc.vector.memset(ones_tmp, 1.0)
        ones_p = singles.tile([128, 1], bf16)
        nc.vector.tensor_copy(ones_p, ones_tmp[:, :1])
        ones_f = singles.tile([1, 128], f32r)
        nc.vector.tensor_copy(ones_f, ones_tmp[:1, :])

        # persistent activations (hd on partitions for q/k; seq for v)
        q_sb = singles.tile([128, QH, S], bf16)
        kt_sb = singles.tile([128, KVH, S], bf16)
        v_sb = singles.tile([128, KVH, NSC, HD], bf16)

        def rope_to(rope_pool, out_full, h_idx, ssl, psum):
            """roped = rope(psum); write bf16 into out_full[:, h_idx, ssl].
            ss_sb's lower half is pre-negated so one full-width add
            finishes the rotation."""
            cc = cc_sb[:, ssl]
            ss = ss_sb[:, ssl]
            t1 = rope_pool.tile([128, QT], f32, name="rope_t1")
            nc.vector.tensor_mul(t1, psum, cc)
            t2 = rope_pool.tile([128, QT], f32, name="rope_t2")
            nc.vector.tensor_mul(t2[:64], psum[64:], ss[:64])
            nc.vector.tensor_mul(t2[64:], psum[:64], ss[64:])
            nc.vector.tensor_add(out_full[:, h_idx, ssl], t1, t2)

        # ---------------- Phase A: K/V projection (half-slab streamed) ------
        with contextlib.ExitStack() as ph:
            rope_a = ph.enter_context(tc.tile_pool(name="ropea", bufs=3))
            wkv_pool = ph.enter_context(tc.tile_pool(name="wkv", bufs=1))
            hs_pool = ph.enter_context(tc.tile_pool(name="hisa", bufs=3))
            ps_k = ph.enter_context(tc.tile_pool(name="ps_k", bufs=4, space="PSUM"))
            ps_v = ph.enter_context(tc.tile_pool(name="ps_v", bufs=4, space="PSUM"))
            HC = DMC // 2

            wk_sb = wkv_pool.tile([128, DMC, FKV], bf16)
            wv_sb = wkv_pool.tile([128, DMC, FKV], bf16)

            for st in range(NQT):
                ssl = slice(st * QT, (st + 1) * QT)
                slabs = []
                for hh in range(2):
                    if st == 0:
                        for c in range(hh * HC, hh * HC + HC):
                            nc.scalar.dma_start(
                                out=wk_sb[:, c, :], in_=wk_r[:, c, :]
                            )
                            nc.scalar.dma_start(
                                out=wv_sb[:, c, :], in_=wv_r[:, c, :]
                            )
                    slab = hs_pool.tile([128, HC, QT], bf16, name="hi_s")
                    nc.sync.dma_start(
                        out=slab, in_=hiT_r[:, hh * HC : (hh + 1) * HC, ssl]
                    )
                    slabs.append(slab)
                pks = [ps_k.tile([128, QT], f32, name="pk") for _ in range(KVH)]
                pvs = [
                    ps_v.tile([128, KVH, HD], f32, name="pv")
                    for _ in range(QT // 128)
                ]
                for c in range(DMC):
                    hi_c = slabs[c // HC][:, c % HC, :]
                    for kv in range(KVH):
                        nc.tensor.matmul(
                            pks[kv],
                            wk_sb[:, c, kv * 128 : (kv + 1) * 128],
                            hi_c,
                            start=(c == 0),
                            stop=(c == DMC - 1),
                        )
                    for j in range(QT // 128):
                        nc.tensor.matmul(
                            pvs[j],
                            hi_c[:, j * 128 : (j + 1) * 128],
                            wv_sb[:, c, :],
                            start=(c == 0),
                            stop=(c == DMC - 1),
                        )
                if st == 0:
                    nc.scalar.dma_start(out=cc_sb, in_=ccT[:, :])
                    nc.scalar.dma_start(out=ss_sb, in_=ssT[:, :])
                    nc.scalar.dma_start(out=tri_sb, in_=tri[:, :])
                for kv in range(KVH):
                    rope_to(rope_a, kt_sb, kv, ssl, pks[kv])
                for j in range(QT // 128):
                    nc.scalar.copy(v_sb[:, :, st * (QT // 128) + j, :], pvs[j])

        # ---------------- Phase B: Q projection (half-head passes) ----------
        with contextlib.ExitStack() as ph:
            rope_b = ph.enter_context(tc.tile_pool(name="ropeb", bufs=3))
            wq_pool = ph.enter_context(tc.tile_pool(name="wqp", bufs=1))
            hs_pool = ph.enter_context(tc.tile_pool(name="his", bufs=3))
            ps_q = ph.enter_context(tc.tile_pool(name="ps_q", bufs=8, space="PSUM"))
            HC = DMC // 2

            wq_sb = wq_pool.tile([128, DMC, FQ], bf16)
            for c in range(DMC):
                nc.scalar.dma_start(out=wq_sb[:, c, :], in_=wq_r[:, c, :])

            for st in range(NQT):
                ssl = slice(st * QT, (st + 1) * QT)
                slabs = []
                for hh in range(2):
                    slab = hs_pool.tile([128, HC, QT], bf16, name="hi_s")
                    nc.sync.dma_start(
                        out=slab, in_=hiT_r[:, hh * HC : (hh + 1) * HC, ssl]
                    )
                    slabs.append(slab)
                for hg in range(2):
                    banks = [
                        ps_q.tile([128, QT], f32, name="pq") for _ in range(QH // 2)
                    ]
                    for c in range(DMC):
                        for b, h in enumerate(range(hg * 4, hg * 4 + 4)):
                            nc.tensor.matmul(
                                banks[b],
                                wq_sb[:, c, h * 128 : (h + 1) * 128],
                                slabs[c // HC][:, c % HC, :],
                                start=(c == 0),
                                stop=(c == DMC - 1),
                            )
                    for b, h in enumerate(range(hg * 4, hg * 4 + 4)):
                        rope_to(rope_b, q_sb, h, ssl, banks[b])

        # ---------------- Phase C+D: attention + output projection ----------
        # The per-head normalization chain (pred -> DRAM transpose -> recip
        # -> broadcast matmul -> scale) has ~15us of DMA latency. Emitting it
        # inline stalls the PE, so each head's tail (broadcast + normalize)
        # is deferred ~2 heads via a closure queue; ctx is evacuated
        # unnormalized to SBUF so its PSUM bank frees immediately.
        with contextlib.ExitStack() as ph:
            wo_pool = ph.enter_context(tc.tile_pool(name="wop", bufs=1))
            ctx_pool = ph.enter_context(tc.tile_pool(name="ctxp", bufs=1))
            pt_pool = ph.enter_context(tc.tile_pool(name="ptp", bufs=4))
            ep_pool = ph.enter_context(tc.tile_pool(name="epp", bufs=4))
            cu_pool = ph.enter_context(tc.tile_pool(name="cup", bufs=4))
            bc_pool = ph.enter_context(tc.tile_pool(name="bcp", bufs=2))
            o_pool = ph.enter_context(tc.tile_pool(name="op", bufs=3))
            ps_a = ph.enter_context(tc.tile_pool(name="ps_a", bufs=4, space="PSUM"))
            ps_ctx = ph.enter_context(tc.tile_pool(name="ps_ctx", bufs=2, space="PSUM"))
            ps_pred = ph.enter_context(tc.tile_pool(name="ps_pred", bufs=2, space="PSUM"))

            wo_sb = wo_pool.tile([128, QH, D], bf16)
            nc.scalar.dma_start(out=wo_sb, in_=wo_r)
            ctx_sb = ctx_pool.tile([128, QH, S], bf16)

            deferred = []

            def emit_head(qt, h):
                nk = G * (qt + 1)
                kv = h // G
                pctx = ps_ctx.tile([128, QT], f32, name="pctx")
                ppred = ps_pred.tile([1, QT], f32, name="ppred")
                for i in range(nk):
                    t = i - G * qt
                    qoff = t * 128 if t >= 0 else 0
                    pss = ps_a.tile([128, QT], f32, name="pss")
                    nc.tensor.matmul(
                        pss[:, qoff:],
                        kt_sb[:, kv, i * 128 : (i + 1) * 128],
                        q_sb[:, h, qt * QT + qoff : (qt + 1) * QT],
                        start=True,
                        stop=True,
                    )
                    pt = pt_pool.tile([128, QT], bf16, name="pt")
                    nc.scalar.activation(
                        pt[:, qoff:], pss[:, qoff:], AF.Exp, scale=_SCALE
                    )
                    if t >= 0:
                        nc.vector.tensor_mul(
                            pt[:, qoff : qoff + 128],
                            pt[:, qoff : qoff + 128],
                            tri_sb,
                        )
                    nc.tensor.matmul(
                        ppred[:, qoff:],
                        ones_p,
                        pt[:, qoff:],
                        start=(i == 0),
                        stop=(i == nk - 1),
                    )
                    nc.tensor.matmul(
                        pctx[:, qoff:],
                        v_sb[:, kv, i, :],
                        pt[:, qoff:],
                        start=(i == 0),
                        stop=(i == nk - 1),
                    )
                # evacuate unnormalized ctx; launch denom->recip chain
                ctxu = cu_pool.tile([128, QT], bf16, name="ctxu")
                nc.scalar.copy(ctxu, pctx)
                slot = (qt * QH + h) % 4
                pred_sb = ep_pool.tile([1, QT], f32, name="pred_sb")
                nc.scalar.copy(pred_sb, ppred)
                nc.sync.dma_start(out=pred_d[slot, :], in_=pred_sb)
                predT = ep_pool.tile([128, NQT], f32, name="predT")
                nc.sync.dma_start(
                    out=predT,
                    in_=pred_d[slot, :].rearrange("(j p) -> p j", p=128),
                )
                recipT = ep_pool.tile([128, NQT], f32r, name="recipT")
                with nc.allow_low_precision(reason="f32r recip: 1e-4 ok here"):
                    nc.vector.reciprocal(recipT, predT)
                nc.sync.dma_start(
                    out=recip_d[slot, :].rearrange("(j p) -> p j", p=128),
                    in_=recipT,
                )
                rrow = ep_pool.tile([1, QT], f32r, name="rrow")
                nc.sync.dma_start(out=rrow, in_=recip_d[slot, :])

                def tail():
                    pbc = ps_a.tile([128, QT], f32, name="pss")
                    nc.tensor.matmul(pbc, ones_f, rrow, start=True, stop=True)
                    bc_sb = bc_pool.tile([128, QT], bf16, name="bc_sb")
                    nc.scalar.copy(bc_sb, pbc)
                    nc.vector.tensor_mul(
                        ctx_sb[:, h, qt * QT : (qt + 1) * QT], ctxu, bc_sb
                    )

                return tail

            def emit_outproj(qt):
                def run():
                    for j in range(QT // 128):
                        sc_idx = qt * (QT // 128) + j
                        for dt in range(NDT):
                            po = ps_a.tile([128, DT], f32, name="pss")
                            for h in range(QH):
                                nc.tensor.matmul(
                                    po,
                                    ctx_sb[:, h, sc_idx * 128 : (sc_idx + 1) * 128],
                                    wo_sb[:, h, dt * DT : (dt + 1) * DT],
                                    start=(h == 0),
                                    stop=(h == QH - 1),
                                )
                            o_sb = o_pool.tile([128, DT], f32, name="o_sb")
                            nc.scalar.copy(o_sb, po)
                            nc.sync.dma_start(
                                out=out[
                                    sc_idx * 128 : (sc_idx + 1) * 128,
                                    dt * DT : (dt + 1) * DT,
                                ],
                                in_=o_sb,
                            )

                return run

            for qt in range(NQT):
                for h in range(QH):
                    deferred.append(emit_head(qt, h))
                    while len(deferred) > 3:
                        deferred.pop(0)()
                deferred.append(emit_outproj(qt))
            while deferred:
                deferred.pop(0)()

    _legalize_waits(nc)
    return nc


_NC_CACHE = {}
_last_exec_ns = None


def _get_nc():
    if "nc" not in _NC_CACHE:
        _NC_CACHE["nc"] = _build_nc()
    return _NC_CACHE["nc"]


# ---------------------------------------------------------------------------
# Optional NTFF profiling hook (used by the local test harness via
# KERNEL_TRACE=1; grading path leaves it off)
# ---------------------------------------------------------------------------
def _install_ntff_hook(so_path="/opt/axon/libaxon_pjrt.so"):
    if "antenv.axon_hooks" in sys.modules:
        return
    try:
        lib = ctypes.CDLL(so_path)
    except OSError:
        lib = None
    if lib is None or not hasattr(lib, "axon_start_nrt_profile"):
        hook = None
    else:
        lib.axon_start_nrt_profile.argtypes = [
            ctypes.POINTER(ctypes.c_int64),
            ctypes.c_size_t,
        ]
        lib.axon_start_nrt_profile.restype = ctypes.c_int64
        lib.axon_stop_nrt_profile.argtypes = [ctypes.c_char_p]
        lib.axon_stop_nrt_profile.restype = ctypes.c_int64

        @contextlib.contextmanager
        def hook(output_dir, device_ids):
            import jax

            jax.devices()
            if device_ids:
                ids = (ctypes.c_int64 * len(device_ids))(*device_ids)
                rc = lib.axon_start_nrt_profile(ids, len(device_ids))
            else:
                rc = lib.axon_start_nrt_profile(None, 0)
            if rc != 0:
                raise RuntimeError(f"axon_start_nrt_profile rc={rc}")
            try:
                yield
            finally:
                n = lib.axon_stop_nrt_profile(str(output_dir).encode())
                print(f"ntff profile: {n} file(s) -> {output_dir}", file=sys.stderr)

    mod = types.ModuleType("antenv.axon_hooks")
    mod.get_axon_ntff_profile_hook = lambda: hook
    sys.modules["antenv.axon_hooks"] = mod


# ---------------------------------------------------------------------------
# Host entry point
# ---------------------------------------------------------------------------
def kernel(hidden_states, position_ids, attention_mask, Wq, Wk, Wv, Wo):
    global _last_exec_ns
    from concourse import bass_utils

    hidden_states = np.asarray(hidden_states, dtype=np.float32)
    position_ids = np.asarray(position_ids)
    attention_mask = np.asarray(attention_mask)
    Wq = np.asarray(Wq, dtype=np.float32)
    Wk = np.asarray(Wk, dtype=np.float32)
    Wv = np.asarray(Wv, dtype=np.float32)
    Wo = np.asarray(Wo, dtype=np.float32)

    if not np.all(np.asarray(attention_mask) > 0):
        # Spec guarantees an all-ones mask; fall back to a host reference
        # implementation for the general case rather than mis-computing.
        return _host_reference(
            hidden_states, position_ids, attention_mask, Wq, Wk, Wv, Wo
        )

    # rope tables per batch: cc/ss [HD, S] with halves stacked
    half = HD // 2
    inv_freq = 1.0 / (THETA ** (np.arange(0, half, dtype=np.float32) / half))
    ccs, sss = [], []
    for b in range(B):
        freqs = position_ids[b].astype(np.float32)[:, None] * inv_freq[None, :]
        cosT = np.cos(freqs).T.astype(np.float32)  # [64, S]
        sinT = np.sin(freqs).T.astype(np.float32)
        ccs.append(np.ascontiguousarray(np.concatenate([cosT, cosT], axis=0)))
        sss.append(np.ascontiguousarray(np.concatenate([-sinT, sinT], axis=0)))

    # causal triangle for aligned diagonal 128-blocks: allowed iff q >= k
    kk = np.arange(SC)[:, None]
    qq = np.arange(SC)[None, :]
    tri = np.ascontiguousarray((qq >= kk).astype(BF16))

    hiTs = [np.ascontiguousarray(hidden_states[b].T.astype(BF16)) for b in range(B)]

    in_maps = []
    for c in range(NCORES):
        b = c // KV_SHARDS
        m = c % KV_SHARDS
        qcols = slice(m * FQ, (m + 1) * FQ)
        kvcols = slice(m * FKV, (m + 1) * FKV)
        in_maps.append(
            {
                "hiT": hiTs[b],
                "wq": np.ascontiguousarray(Wq[:, qcols].astype(BF16)),
                "wk": np.ascontiguousarray(Wk[:, kvcols].astype(BF16)),
                "wv": np.ascontiguousarray(Wv[:, kvcols].astype(BF16)),
                "wo": np.ascontiguousarray(Wo[qcols, :].astype(BF16)),
                "ccT": ccs[b],
                "ssT": sss[b],
                "tri": tri,
            }
        )

    nc = _get_nc()
    trace = os.environ.get("KERNEL_TRACE", "") == "1"
    if trace:
        _install_ntff_hook()
        bass_utils.upload_artifacts = lambda tmpdir: f"local:{tmpdir}"
    res = bass_utils.run_bass_kernel_spmd(
        nc, in_maps, list(range(NCORES)), trace=trace
    )
    _last_exec_ns = res.exec_time_ns

    out = np.zeros((B, S, D), dtype=np.float32)
    for c in range(NCORES):
        out[c // KV_SHARDS] += res.results[c]["out"]
    return out


def _host_reference(hidden_states, position_ids, attention_mask, Wq, Wk, Wv, Wo):
    """Numpy fallback for inputs outside the spec's guarantees."""
    q = (hidden_states @ Wq).reshape(B, S, H, HD)
    k = (hidden_states @ Wk).reshape(B, S, HKV, HD)
    v = (hidden_states @ Wv).reshape(B, S, HKV, HD)

    half = HD // 2
    inv_freq = 1.0 / (THETA ** (np.arange(0, half, dtype=np.float32) / half))
    freqs = position_ids.astype(np.float32)[..., None] * inv_freq
    cos = np.cos(freqs)[:, :, None, :]
    sin = np.sin(freqs)[:, :, None, :]

    def rope(x):
        x1, x2 = x[..., :half], x[..., half:]
        return np.concatenate([x1 * cos - x2 * sin, x2 * cos + x1 * sin], axis=-1)

    q, k = rope(q), rope(k)
    qg = q.reshape(B, S, HKV, G, HD)
    scores = np.einsum("bqhgd,bkhd->bhgqk", qg, k) * (HD**-0.5)
    causal = np.tril(np.ones((S, S), bool))
    mask = causal[None, None, None] & (attention_mask[:, None, None, None, :] > 0)
    scores = np.where(mask, scores, np.finfo(np.float32).min)
    scores = scores - scores.max(axis=-1, keepdims=True)
    probs = np.exp(scores)
    probs = probs / probs.sum(axis=-1, keepdims=True)
    ctx = np.einsum("bhgqk,bkhd->bqhgd", probs, v).reshape(B, S, H * HD)
    return (ctx @ Wo).astype(np.float32)
